# revision 18
# baseline (speedup 1.0000x reference)
"""Trainium2 Bass kernel for one transformer block (nn_Block_25838523252853).

Full inputs in, full output out. Sharding: the 4096 tokens (B=4 x L=1024)
are split 8 ways -- each core owns 512 tokens (half of one sequence).
Attention needs full-sequence K/V, so cores exchange their K/V halves with
their pair-neighbour via two AllGathers, split by head group so attention
on heads 0-7 starts while the second gather is in flight.

Device layout: activations are channel-major bf16 ([C_part, T_free] tiles),
weights in natural [inC, outC] layout as matmul lhsT. LayerNorm reductions
(over channels = partitions) use ones-vector matmuls interleaved with the
producing matmuls; per-token scalars are broadcast across partitions with
tiny K=1/K=2 matmuls whose lhsT rows carry gamma/beta, so the normalize
epilogue is two fused DVE ops per tile and the scalar engine only runs
Exp/Gelu and four tiny LN-chain ops. Softmax skips the max subtraction
(scores are bounded ~|9| for this problem) and gets row sums free from a
ones-column appended to V. fc2 runs k-outer over all 8 PSUM banks with w2
streamed through 4 rotating SBUF tiles (never fully resident).
"""

import numpy as np
import ml_dtypes

import concourse.bass as bass
import concourse.tile as tile
from concourse import bacc, mybir
from concourse.bass_utils import run_bass_kernel_spmd

F32 = mybir.dt.float32
BF16 = mybir.dt.bfloat16
FP16 = mybir.dt.float16

DIM = 1024
HEADS = 16
HD = 64
HIDDEN = 4096
EPS = 1e-5
SCALE = HD ** -0.5
B, L = 4, 1024
T = 512          # tokens owned per core
P = 128
NC = 8

_BUILT = None

AF = mybir.ActivationFunctionType
ALU = mybir.AluOpType


def build():
    """Build + bacc-compile the SPMD program. Cached per process."""
    global _BUILT
    if _BUILT is not None:
        return _BUILT

    nc = bacc.Bacc("TRN2", target_bir_lowering=False, debug=False, num_devices=NC)

    d_xs = nc.dram_tensor("xs", [2 * DIM, T], BF16, kind="ExternalInput").ap()
    d_wsk = nc.dram_tensor("wsk", [2 * DIM, DIM], BF16, kind="ExternalInput").ap()
    d_wq = nc.dram_tensor("wq", [DIM, DIM], BF16, kind="ExternalInput").ap()
    d_wk = nc.dram_tensor("wk", [DIM, DIM], BF16, kind="ExternalInput").ap()
    d_wv = nc.dram_tensor("wv", [DIM, DIM], BF16, kind="ExternalInput").ap()
    d_wp = nc.dram_tensor("wp", [DIM, DIM], BF16, kind="ExternalInput").ap()
    d_w1 = nc.dram_tensor("w1", [DIM, HIDDEN], BF16, kind="ExternalInput").ap()
    d_w2 = nc.dram_tensor("w2", [HIDDEN, DIM], BF16, kind="ExternalInput").ap()
    # per-channel columns: [128, 56]: skb 0:8, pb 8:16, b2 16:24, b1 24:56
    d_cols = nc.dram_tensor("cols", [P, 56], F32, kind="ExternalInput").ap()
    # gamma cols for STT epilogues: [128, 24]: g1 0:8, g2 8:16, g3 16:24
    d_gcol = nc.dram_tensor("gcol", [P, 24], F32, kind="ExternalInput").ap()
    # bcast lhsT rows, bf16: [2, 3*DIM]: cols i*DIM+c: row0 = -g_i, row1 = beta_i
    d_gb = nc.dram_tensor("gb", [2, 3 * DIM], BF16, kind="ExternalInput").ap()
    # collective bounce buffers: ccA = K chans 0:512 + V chans 0:512 (heads 0-7)
    ccW_in = nc.dram_tensor("ccW_in", [1, 64], BF16).ap()
    ccW_out = nc.dram_tensor("ccW_out", [2, 64], BF16).ap()
    ccA_in = nc.dram_tensor("ccA_in", [DIM, T], BF16).ap()
    ccA_out = nc.dram_tensor("ccA_out", [2 * DIM, T], BF16).ap()
    ccB_in = nc.dram_tensor("ccB_in", [DIM, T], BF16).ap()
    ccB_out = nc.dram_tensor("ccB_out", [2 * DIM, T], BF16).ap()
    d_out = nc.dram_tensor("out", [DIM, T], F32, kind="ExternalOutput").ap()

    GROUPS = [[0, 1], [2, 3], [4, 5], [6, 7]]
    EXPW = 2 * T

    with tile.TileContext(nc, pool_alloc_mode="queue") as tc:
        with tc.tile_pool(name="glob", bufs=1) as gpool, \
             tc.tile_pool(name="tmp", bufs=2) as tpool, \
             tc.tile_pool(name="ps", bufs=3, space="PSUM") as ppool:

            # warm-up AllGather: pays the one-time ~10us CC mesh startup
            # during phase A instead of on the critical first real gather
            nc.gpsimd.collective_compute(
                "AllGather", ALU.bypass,
                replica_groups=[[0, 1], [2, 3], [4, 5], [6, 7]],
                ins=[ccW_in[:]], outs=[ccW_out[:]],
            )
            cols = gpool.tile([P, 56], F32, tag="cols", name="cols")
            gcol = gpool.tile([P, 24], F32, tag="gcol", name="gcol")
            gb = gpool.tile([2, 3 * DIM], BF16, tag="gb", name="gb")
            ones_b = gpool.tile([P, 1], BF16, tag="ones_b", name="ones_b")
            nc.vector.memset(ones_b, 1.0)
            ones_h = gpool.tile([P, 1], FP16, tag="ones_h", name="ones_h")
            nc.vector.memset(ones_h, 1.0)
            ones_row = gpool.tile([1, P], BF16, tag="ones_row", name="ones_row")
            nc.vector.memset(ones_row, 1.0)
            eps_t = gpool.tile([1, 1], F32, tag="eps_t", name="eps_t")
            nc.vector.memset(eps_t, EPS)
            # per-LN broadcast rhs [2, T]: row0 = mean*rstd (written), row1 = 1
            brh = [gpool.tile([2, T], BF16, tag=f"brh{i}", name=f"brh{i}")
                   for i in range(3)]
            for i in range(3):
                # whole-tile memset (1-partition access at base 1 is illegal);
                # row 0 is overwritten with mean*rstd by each LN chain
                nc.vector.memset(brh[i], 1.0)
            # long-lived activation tiles (hT aliases x1n/oT slots later)
            x1n = [gpool.tile([P, T], BF16, tag=f"x1n_{k}", name=f"x1n_{k}")
                   for k in range(8)]
            x2n = [gpool.tile([P, T], BF16, tag=f"x2n{m}", name=f"x2n{m}")
                   for m in range(8)]
            oT = [gpool.tile([P, T], BF16, tag=f"oT{m}", name=f"oT{m}")
                  for m in range(8)]
            wp = [gpool.tile([P, DIM], BF16, tag=f"wp{k}", name=f"wp{k}")
                  for k in range(8)]

            C_SKB, C_PRB, C_F2B, C_F1B = 0, 8, 16, 24

            def emit_stats(stats, raw_m, sq_m, m, nk=8):
                nc.tensor.matmul(stats[0:1, :], lhsT=ones_b, rhs=raw_m,
                                 start=(m == 0), stop=(m == nk - 1))
                nc.tensor.matmul(stats[32:33, :], lhsT=ones_h, rhs=sq_m,
                                 start=(m == 0), stop=(m == nk - 1))

            def emit_ln_chain(stats, ln_i, n_feat):
                """LN stats -> (a_sb [128,T] bf16 = rstd bcast, make_b(m) fn)."""
                inv_n = 1.0 / n_feat
                msq = tpool.tile([1, T], F32, tag="ln_msq", name="ln_msq", bufs=1)
                nc.scalar.activation(msq, stats[0:1, :], AF.Square, scale=inv_n)
                var = tpool.tile([1, T], F32, tag="ln_var", name="ln_var", bufs=1)
                nc.vector.scalar_tensor_tensor(var, stats[32:33, :], inv_n, msq,
                                               ALU.mult, ALU.subtract)
                lnv = tpool.tile([1, T], F32, tag="ln_lnv", name="ln_lnv", bufs=1)
                nc.scalar.activation(lnv, var, AF.Ln, bias=eps_t)
                rstd = tpool.tile([1, T], F32, tag="ln_rstd", name="ln_rstd",
                                  bufs=1)
                nc.scalar.activation(rstd, lnv, AF.Exp, scale=-0.5)
                rstd_bf = tpool.tile([1, T], BF16, tag="ln_rstdb", name="ln_rstdb",
                                     bufs=1)
                nc.vector.tensor_copy(out=rstd_bf, in_=rstd)
                # brh row0 = mean * rstd  (bf16)
                nc.vector.scalar_tensor_tensor(brh[ln_i][0:1, :], stats[0:1, :],
                                               inv_n, rstd, ALU.mult, ALU.mult)
                a_ps = ppool.tile([P, T], F32, tag="mm", name="mm")
                nc.tensor.matmul(a_ps, lhsT=ones_row, rhs=rstd_bf, start=True,
                                 stop=True)
                a_sb = tpool.tile([P, T], BF16, tag="ln_asb", name="ln_asb", bufs=1)
                nc.vector.tensor_copy(out=a_sb, in_=a_ps)

                def make_b(m):
                    b_ps = ppool.tile([P, T], F32, tag="mm", name="mm")
                    nc.tensor.matmul(b_ps, lhsT=gb[:, ln_i * DIM + m * P:
                                                   ln_i * DIM + (m + 1) * P],
                                     rhs=brh[ln_i], start=True, stop=True)
                    return b_ps
                return a_sb, make_b

            def emit_ln_final(raw_m, a_sb, b_ps, ln_i, m, out_tile):
                """out = g*(raw*rstd) + (-g*mr + beta), two DVE ops."""
                t1 = tpool.tile([P, T], BF16, tag="ln_t1", name="ln_t1", bufs=2)
                nc.vector.tensor_tensor(t1, raw_m, a_sb, ALU.mult)
                nc.vector.scalar_tensor_tensor(
                    out_tile, t1, gcol[:, 8 * ln_i + m:8 * ln_i + m + 1], b_ps,
                    ALU.mult, ALU.add)

            # =========== Phase A: skip-concat linear + LN1 stats ===========
            wqkvp = tc.alloc_tile_pool(name="wqkv", bufs=1)
            with tc.tile_pool(name="pha", bufs=1) as apool:
                wsk = [apool.tile([P, DIM], BF16, tag=f"wsk{k}", name=f"wsk{k}")
                       for k in range(16)]
                xs = [apool.tile([P, T], BF16, tag=f"xsh{k}", name=f"xsh{k}")
                      for k in range(16)]
                for k in range(16):
                    eng = nc.sync if k % 2 == 0 else nc.gpsimd
                    eng.dma_start(out=wsk[k], in_=d_wsk[k * P:(k + 1) * P, :])
                    eng.dma_start(out=xs[k], in_=d_xs[k * P:(k + 1) * P, :])
                    if k == 0:
                        nc.gpsimd.dma_start(out=cols, in_=d_cols)
                        nc.gpsimd.dma_start(out=gcol, in_=d_gcol)
                        nc.gpsimd.dma_start(out=gb, in_=d_gb)
                # prefetch q/k/v weights behind phase-A tiles
                wq = [wqkvp.tile([P, DIM], BF16, tag=f"wq{k}", name=f"wq{k}")
                      for k in range(8)]
                wk = [wqkvp.tile([P, DIM], BF16, tag=f"wk{k}", name=f"wk{k}")
                      for k in range(8)]
                wv = [wqkvp.tile([P, DIM], BF16, tag=f"wv{k}", name=f"wv{k}")
                      for k in range(8)]
                for k in range(8):
                    nc.sync.dma_start(out=wk[k], in_=d_wk[k * P:(k + 1) * P, :])
                    nc.gpsimd.dma_start(out=wv[k], in_=d_wv[k * P:(k + 1) * P, :])
                for k in range(8):
                    nc.sync.dma_start(out=wq[k], in_=d_wq[k * P:(k + 1) * P, :])

                raw = [apool.tile([P, T], BF16, tag=f"raw{m}", name=f"raw{m}")
                       for m in range(8)]
                sq = [apool.tile([P, T], FP16, tag=f"sq{m}", name=f"sq{m}")
                      for m in range(8)]
                # hybrid k-outer: ride the input DMA stream with all 8 psum
                # slots, then finish per-m so the drains stagger
                aslots = [ppool.tile([P, EXPW], F32, tag="big", bufs=2,
                                     name="abig") for _ in range(2)]
                aslots += [ppool.tile([P, T], F32, tag="mm", name="amm")
                           for _ in range(3)]
                aslots += [ppool.tile([P, T], F32, tag="st", name="ast", bufs=1)]

                def aslot(m):
                    if m < 4:
                        return aslots[m // 2][:, (m % 2) * T:(m % 2 + 1) * T]
                    return aslots[2 + (m - 4)]

                for k in range(12):
                    for m in range(8):
                        nc.tensor.matmul(
                            aslot(m), lhsT=wsk[k][:, m * P:(m + 1) * P],
                            rhs=xs[k], start=(k == 0), stop=False)
                stats1 = None
                for m in range(8):
                    for k in range(12, 16):
                        nc.tensor.matmul(
                            aslot(m), lhsT=wsk[k][:, m * P:(m + 1) * P],
                            rhs=xs[k], start=False, stop=(k == 15))
                    # raw = psum + skip_b (per-partition col), bf16
                    nc.vector.tensor_scalar(raw[m], aslot(m),
                                            cols[:, C_SKB + m:C_SKB + m + 1], None,
                                            ALU.add)
                    nc.scalar.activation(sq[m], raw[m], AF.Square)
                    if m == 7:
                        stats1 = ppool.tile([P, T], F32, tag="st", name="st1",
                                            bufs=1)
                    if stats1 is not None:
                        if m == 7:
                            for m2 in range(8):
                                emit_stats(stats1, raw[m2], sq[m2], m2)

                # ---- LN1 chain + finals (finals pipeline into K below) ----
                a1, make_b1 = emit_ln_chain(stats1, 0, DIM)
                for m in range(8):
                    emit_ln_final(raw[m], a1, make_b1(m), 0, m, x1n[m])

            # =========== Phase B: K, V halves + AllGathers, Q ===========
            with tc.tile_pool(name="phb", bufs=1) as bpool:
                cc_in = [ccA_in, ccB_in]
                cc_out = [ccA_out, ccB_out]
                kloc = [bpool.tile([P, T], BF16, tag=f"kl{m}", name=f"kl{m}")
                        for m in range(8)]
                vloc = [bpool.tile([P, T], BF16, tag=f"vl{i}", name=f"vl{i}")
                        for i in range(8)]
                qT = [bpool.tile([P, T], BF16, tag=f"qT{m}", name=f"qT{m}")
                      for m in range(8)]
                for half in (0, 2, 1):
                    if half == 2:
                        # Q for own tokens (overlaps AG-A), m-outer
                        for m in range(8):
                            ps = ppool.tile([P, T], F32, tag="mm", name="mm")
                            for k in range(8):
                                nc.tensor.matmul(
                                    ps, lhsT=wq[k][:, m * P:(m + 1) * P],
                                    rhs=x1n[k], start=(k == 0), stop=(k == 7))
                            nc.vector.tensor_copy(out=qT[m], in_=ps)
                        continue
                    if half == 0:
                        # K half0 k-outer: pipelines with the LN1 finals
                        pk = [ppool.tile([P, EXPW], F32, tag="big", bufs=2,
                                         name="kbig") for _ in range(2)]
                        for k in range(8):
                            for mi in range(4):
                                nc.tensor.matmul(
                                    pk[mi // 2][:, (mi % 2) * T:(mi % 2 + 1) * T],
                                    lhsT=wk[k][:, mi * P:(mi + 1) * P],
                                    rhs=x1n[k], start=(k == 0), stop=(k == 7))
                        for mi in range(4):
                            nc.vector.tensor_copy(
                                out=kloc[mi],
                                in_=pk[mi // 2][:, (mi % 2) * T:(mi % 2 + 1) * T])
                            nc.sync.dma_start(
                                out=cc_in[0][mi * P:(mi + 1) * P, :],
                                in_=kloc[mi])
                    else:
                        for mi in range(4):
                            m = 4 + mi
                            ps = ppool.tile([P, T], F32, tag="mm", name="mm")
                            for k in range(8):
                                nc.tensor.matmul(
                                    ps, lhsT=wk[k][:, m * P:(m + 1) * P],
                                    rhs=x1n[k], start=(k == 0), stop=(k == 7))
                            nc.vector.tensor_copy(out=kloc[m], in_=ps)
                            nc.sync.dma_start(
                                out=cc_in[1][mi * P:(mi + 1) * P, :],
                                in_=kloc[m])
                    # V token-major: out [tok128, 512chan-half], m-outer
                    for kt in range(4):
                        ps = ppool.tile([P, T], F32, tag="mm", name="mm")
                        for k in range(8):
                            nc.tensor.matmul(
                                ps, lhsT=x1n[k][:, kt * P:(kt + 1) * P],
                                rhs=wv[k][:, half * T:(half + 1) * T],
                                start=(k == 0), stop=(k == 7))
                        nc.vector.tensor_copy(out=vloc[half * 4 + kt], in_=ps)
                        nc.sync.dma_start(
                            out=cc_in[half][T + kt * P:T + (kt + 1) * P, :],
                            in_=vloc[half * 4 + kt])
                    nc.gpsimd.collective_compute(
                        "AllGather", ALU.bypass, replica_groups=GROUPS,
                        ins=[cc_in[half][:]], outs=[cc_out[half][:]],
                    )

                # reload gathered K/V (uniform across cores; k-token order is
                # attention-invariant). kT[m][b]: chans m*128.., token block b.
                kT = [[bpool.tile([P, T], BF16, tag=f"kT_{m}_{b}",
                                  name=f"kT_{m}_{b}")
                       for b in range(2)] for m in range(8)]
                # per-half V tiles: heads 0-7 tiles depend only on AG-A so
                # attention on heads 0-7 never waits for AG-B
                v_hf = [[bpool.tile([P, 8 * (HD + 1)], BF16, tag=f"v{half}_{kt}",
                                    name=f"v{half}_{kt}")
                         for kt in range(8)] for half in range(2)]
                for half in range(2):
                    for kt in range(8):
                        v3 = v_hf[half][kt].rearrange("p (h c) -> p h c", c=HD + 1)
                        nc.vector.memset(v3[:, :, HD:HD + 1], 1.0)
                for half in range(2):
                    for b in range(2):
                        for mi in range(4):
                            m = half * 4 + mi
                            nc.sync.dma_start(
                                out=kT[m][b],
                                in_=cc_out[half][b * DIM + mi * P:
                                                 b * DIM + (mi + 1) * P, :])
                        for ktl in range(4):
                            kt = b * 4 + ktl
                            v3 = v_hf[half][kt].rearrange("p (h c) -> p h c",
                                                          c=HD + 1)
                            nc.sync.dma_start(
                                out=v3[:, :, 0:HD],
                                in_=cc_out[half][b * DIM + T + ktl * P:
                                                 b * DIM + T + (ktl + 1) * P, :]
                                .rearrange("p (h c) -> p h c", c=HD))

                for k in range(8):
                    nc.gpsimd.dma_start(out=wp[k], in_=d_wp[k * P:(k + 1) * P, :])

                # =========== Phase C: attention ===========
                with tc.tile_pool(name="exps", bufs=7) as xpool:
                    # per-head normalization straight off the PSUM sums row:
                    # rh = 1/sum (DVE approx), bc = broadcast matmul, then
                    # oT = po * bc with both operands still in PSUM.
                    prev = None

                    def finish_head(hd, oUb, rhb):
                        m2b, offb = hd // 2, (hd % 2) * HD
                        bc = ppool.tile([P, T], F32, tag="mm", name="mm")
                        nc.tensor.matmul(bc[0:HD, :], lhsT=ones_row[:, 0:HD],
                                         rhs=rhb, start=True, stop=True)
                        nc.vector.tensor_tensor(oT[m2b][offb:offb + HD, :],
                                                oUb, bc[0:HD, :], ALU.mult)

                    heat = ppool.tile([P, T], F32, tag="st", name="heat", bufs=1)
                    n_heat = 0
                    for hd in range(HEADS):
                        m2, off = hd // 2, (hd % 2) * HD
                        vh, hh = v_hf[hd // 8], hd % 8
                        exp_tiles = []
                        for kp in range(4):
                            ps2 = ppool.tile([P, EXPW], F32, tag="big", bufs=2,
                                             name="mm2")
                            for j in range(2):
                                kt = 2 * kp + j
                                th, col = kt // 4, (kt % 4) * P
                                nc.tensor.matmul(
                                    ps2[:, j * T:(j + 1) * T],
                                    lhsT=kT[m2][th][off:off + HD, col:col + P],
                                    rhs=qT[m2][off:off + HD, :], start=True,
                                    stop=True)
                            e = xpool.tile([P, EXPW], BF16, tag="exp", name="exp")
                            nc.scalar.activation(e, ps2, AF.Exp)
                            exp_tiles.append(e)
                        if prev is not None:
                            finish_head(*prev)
                        # dead matmuls: hold the PE HAM clock at 2.4 GHz
                        # through the ACT-bound stretch
                        for _ in range(4):
                            nc.tensor.matmul(heat, lhsT=qT[m2][:, 0:P],
                                             rhs=qT[m2], start=(n_heat == 0),
                                             stop=(hd == HEADS - 1 and _ == 3))
                            n_heat += 1
                        po = ppool.tile([P, T], F32, tag="mm", name="mm")
                        for kt in range(8):
                            nc.tensor.matmul(
                                po[0:HD + 1, :],
                                lhsT=vh[kt][:, hh * (HD + 1):(hh + 1) * (HD + 1)],
                                rhs=exp_tiles[kt // 2][:, (kt % 2) * T:
                                                       (kt % 2 + 1) * T],
                                start=(kt == 0), stop=(kt == 7))
                        sms = tpool.tile([1, T], F32, tag="sms", name="sms",
                                         bufs=1)
                        nc.vector.tensor_copy(out=sms, in_=po[HD:HD + 1, :])
                        rh = tpool.tile([1, T], F32, tag="rh", name="rh", bufs=1)
                        nc.vector.reciprocal_approx_fast(out=rh, in_=sms)
                        rhb = tpool.tile([1, T], BF16, tag="rhb", name="rhb",
                                         bufs=2)
                        nc.vector.tensor_copy(out=rhb, in_=rh)
                        oUb = tpool.tile([HD, T], BF16, tag="oub", name="oub",
                                         bufs=3)
                        nc.vector.tensor_copy(out=oUb, in_=po[0:HD, :])
                        prev = (hd, oUb, rhb)
                    finish_head(*prev)

                    # proj k-outer over 5 psum slots starts while the last
                    # head's epilogue drains
                    pj = [ppool.tile([P, EXPW], F32, tag="big", bufs=2,
                                     name="pjbig") for _ in range(2)]
                    pj += [ppool.tile([P, T], F32, tag="st", name="pjst", bufs=1)]

                    def pjslot(m):
                        if m < 4:
                            return pj[m // 2][:, (m % 2) * T:(m % 2 + 1) * T]
                        return pj[2]

                    for k in range(8):
                        for m in range(5):
                            nc.tensor.matmul(pjslot(m),
                                             lhsT=wp[k][:, m * P:(m + 1) * P],
                                             rhs=oT[k], start=(k == 0),
                                             stop=(k == 7))
            wqkvp.release()

            # =========== Phase D: proj m5..7 + residual + LN2 ===========
            w1pool = tc.alloc_tile_pool(name="w1p", bufs=1)
            w1 = [w1pool.tile([P, HIDDEN], BF16, tag=f"w1{k}", name=f"w1{k}")
                  for k in range(8)]
            for k in range(8):
                eng = nc.sync if k % 2 == 0 else nc.gpsimd
                eng.dma_start(out=w1[k], in_=d_w1[k * P:(k + 1) * P, :])
            with tc.tile_pool(name="phd", bufs=1) as dpool:
                x2r = [dpool.tile([P, T], BF16, tag=f"x2r{m}", name=f"x2r{m}")
                       for m in range(8)]
                x2sq = [dpool.tile([P, T], FP16, tag=f"x2sq{m}", name=f"x2sq{m}")
                        for m in range(8)]
                pj5 = [None] * 3
                for mi in range(3):
                    m = 5 + mi
                    ps = ppool.tile([P, T], F32, tag="mm", name="mm")
                    for k in range(8):
                        nc.tensor.matmul(ps, lhsT=wp[k][:, m * P:(m + 1) * P],
                                         rhs=oT[k], start=(k == 0), stop=(k == 7))
                    pj5[mi] = ps
                stats2 = None
                for m in range(8):
                    ps = pjslot(m) if m < 5 else pj5[m - 5]
                    # x2r = psum + proj_b + x1n  (one fused DVE op)
                    nc.vector.scalar_tensor_tensor(
                        x2r[m], ps, cols[:, C_PRB + m:C_PRB + m + 1], x1n[m],
                        ALU.add, ALU.add)
                    nc.scalar.activation(x2sq[m], x2r[m], AF.Square)
                    if m == 0:
                        stats2 = ppool.tile([P, T], F32, tag="st", name="st2",
                                            bufs=1)
                    emit_stats(stats2, x2r[m], x2sq[m], m)

                a2, make_b2 = emit_ln_chain(stats2, 1, DIM)
                for m in range(8):
                    emit_ln_final(x2r[m], a2, make_b2(m), 1, m, x2n[m])

            # =========== Phase E: MLP + LN3 ===========
            with tc.tile_pool(name="phe", bufs=1) as epool:
                hT = [epool.tile([P, T], BF16, tag=f"hT{i}", name=f"hT{i}")
                      for i in range(32)]
                pf = None
                for mm in range(32):
                    if mm < 4:
                        # k-outer for the first 4 out-tiles (pipelines with LN2)
                        if mm == 0:
                            pf = [ppool.tile([P, EXPW], F32, tag="big", bufs=2,
                                             name="fbig") for _ in range(2)]
                            for k in range(8):
                                for j in range(4):
                                    nc.tensor.matmul(
                                        pf[j // 2][:, (j % 2) * T:(j % 2 + 1) * T],
                                        lhsT=w1[k][:, j * P:(j + 1) * P],
                                        rhs=x2n[k], start=(k == 0), stop=(k == 7))
                        ps = pf[mm // 2][:, (mm % 2) * T:(mm % 2 + 1) * T]
                    else:
                        ps = ppool.tile([P, T], F32, tag="mm", name="mm")
                        for k in range(8):
                            nc.tensor.matmul(ps, lhsT=w1[k][:, mm * P:(mm + 1) * P],
                                             rhs=x2n[k], start=(k == 0), stop=(k == 7))
                    nc.scalar.activation(hT[mm], ps, AF.Gelu,
                                         bias=cols[:, C_F1B + mm:C_F1B + mm + 1])

                # ---- fc2: two k-outer passes of 4 out-tiles, w2 streamed
                # through 4 rotating SBUF tiles (re-read on pass 2); pass-1
                # drains overlap pass-2 matmuls ----
                x3sq = [epool.tile([P, T], FP16, tag=f"x3sq{m}", name=f"x3sq{m}")
                        for m in range(8)]
                x3r = [epool.tile([P, T], BF16, tag=f"x3r{m}", name=f"x3r{m}")
                       for m in range(8)]
                f2ps = {}
                for p2 in range(2):
                    if p2 == 0:
                        slots = [ppool.tile([P, EXPW], F32, tag="big", bufs=2,
                                            name="f2big") for _ in range(2)]
                        sl = lambda m: slots[m // 2][:, (m % 2) * T:
                                                     (m % 2 + 1) * T]
                    else:
                        slots = [ppool.tile([P, T], F32, tag="mm", name="f2mm")
                                 for _ in range(3)]
                        slots += [ppool.tile([P, T], F32, tag="st", name="f2st",
                                             bufs=1)]
                        sl = lambda m: slots[m - 4]
                    for k in range(32):
                        w2t = epool.tile([P, T], BF16, tag=f"w2r{k % 4}",
                                         name=f"w2_{p2}_{k}")
                        eng = nc.sync if k % 2 == 0 else nc.gpsimd
                        eng.dma_start(out=w2t,
                                      in_=d_w2[k * P:(k + 1) * P,
                                               p2 * T:(p2 + 1) * T])
                        for mi in range(4):
                            m = p2 * 4 + mi
                            nc.tensor.matmul(sl(m),
                                             lhsT=w2t[:, mi * P:(mi + 1) * P],
                                             rhs=hT[k], start=(k == 0),
                                             stop=(k == 31))
                    if p2 == 0:
                        # stats live in a freed pass-1 "big" bank so the
                        # stats matmuls interleave with pass-2 matmuls
                        stats3 = ppool.tile([P, EXPW], F32, tag="big", bufs=2,
                                            name="st3")[:, 0:T]
                    for mi in range(4):
                        m = p2 * 4 + mi
                        f2ps[m] = sl(m)
                        nc.vector.scalar_tensor_tensor(
                            x3r[m], f2ps[m], cols[:, C_F2B + m:C_F2B + m + 1],
                            x2n[m], ALU.add, ALU.add)
                        nc.scalar.activation(x3sq[m], x3r[m], AF.Square)
                        emit_stats(stats3, x3r[m], x3sq[m], m)

                a3, make_b3 = emit_ln_chain(stats3, 2, DIM)
                for m in range(8):
                    xo = tpool.tile([P, T], F32, tag="xo", name="xo", bufs=2)
                    emit_ln_final(x3r[m], a3, make_b3(m), 2, m, xo)
                    eng = nc.sync if m % 2 == 0 else nc.gpsimd
                    eng.dma_start(out=d_out[m * P:(m + 1) * P, :], in_=xo)
            w1pool.release()

    # Steer the act-table selector: keep dict ORDER (act_func_set_id is the
    # positional index into act_info.json) but hide Exp/Ln from the small
    # tables so both resolve to the combined natural_log_exp_and_others set
    # and the attention/LN loop stops thrashing table loads.
    import concourse.hw_specs as hw_specs
    tabs = dict(hw_specs.get_activation_tables("gen3"))
    steered = {}
    for name, fns in tabs.items():
        fns = set(fns)
        if name != "natural_log_exp_and_others":
            fns.discard(AF.Exp)
            fns.discard(AF.Ln)
        steered[name] = fns
    import functools
    _orig = hw_specs.get_activation_tables
    patched = functools.lru_cache(None)(
        lambda arch: steered if arch == "gen3" else _orig(arch))
    hw_specs.get_activation_tables = patched
    import concourse.bacc as bacc_mod
    bacc_mod.get_activation_tables = patched

    nc.compile()
    _BUILT = nc
    return nc


def _pack_col(vec, ncols):
    """[N] per-channel vector -> [128, N//128] tile layout (channel c -> [c%128, c//128])."""
    return np.ascontiguousarray(vec.reshape(ncols, P).T.astype(np.float32))


def _prep_in_maps(inputs):
    bf = ml_dtypes.bfloat16
    x = np.asarray(inputs["x"], np.float32)
    skip = np.asarray(inputs["skip"], np.float32)
    xs = np.concatenate([x, skip], axis=2)          # [4, 1024, 2048]

    wsk = np.asarray(inputs["skip_w"], np.float32).astype(bf)
    qkv = np.asarray(inputs["qkv_w"], np.float32)
    wq = (qkv[:, :DIM] * SCALE).astype(bf)
    wk = np.ascontiguousarray(qkv[:, DIM:2 * DIM]).astype(bf)
    wv = np.ascontiguousarray(qkv[:, 2 * DIM:]).astype(bf)
    wp = np.asarray(inputs["proj_w"], np.float32).astype(bf)
    w1 = np.asarray(inputs["fc1_w"], np.float32).astype(bf)
    w2 = np.asarray(inputs["fc2_w"], np.float32).astype(bf)

    cols = np.zeros((P, 56), np.float32)
    cols[:, 0:8] = _pack_col(np.asarray(inputs["skip_b"], np.float32), 8)
    cols[:, 8:16] = _pack_col(np.asarray(inputs["proj_b"], np.float32), 8)
    cols[:, 16:24] = _pack_col(np.asarray(inputs["fc2_b"], np.float32), 8)
    cols[:, 24:56] = _pack_col(np.asarray(inputs["fc1_b"], np.float32), 32)

    gcolv = np.zeros((P, 24), np.float32)
    gcolv[:, 0:8] = _pack_col(np.asarray(inputs["ln1_g"], np.float32), 8)
    gcolv[:, 8:16] = _pack_col(np.asarray(inputs["ln2_g"], np.float32), 8)
    gcolv[:, 16:24] = _pack_col(np.asarray(inputs["ln3_g"], np.float32), 8)

    gbv = np.zeros((2, 3 * DIM), np.float32)
    for i, (gk, bk) in enumerate([("ln1_g", "ln1_b"), ("ln2_g", "ln2_b"),
                                  ("ln3_g", "ln3_b")]):
        gbv[0, i * DIM:(i + 1) * DIM] = -np.asarray(inputs[gk], np.float32)
        gbv[1, i * DIM:(i + 1) * DIM] = np.asarray(inputs[bk], np.float32)

    in_maps = []
    for c in range(NC):
        b, h = c // 2, c % 2
        seq = xs[b][h * T:(h + 1) * T]               # own 512 tokens
        xsT = np.ascontiguousarray(seq.T).astype(bf)  # [2048, 512]
        in_maps.append({
            "xs": xsT, "wsk": wsk, "wq": wq, "wk": wk, "wv": wv,
            "wp": wp, "w1": w1, "w2": w2, "cols": cols, "gcol": gcolv,
            "gb": gbv.astype(bf),
        })
    return in_maps


def run(inputs, trace=False, trace_kwargs=None):
    nc = build()
    in_maps = _prep_in_maps(inputs)
    res = run_bass_kernel_spmd(nc, in_maps, core_ids=list(range(NC)),
                               trace=trace, **(trace_kwargs or {}))
    full = np.empty((B, L, DIM), np.float32)
    for c in range(NC):
        b, h = c // 2, c % 2
        full[b, h * T:(h + 1) * T, :] = res.results[c]["out"].T
    return full, res


def kernel(**inputs):
    out, _ = run(inputs, trace=False)
    return out
